# revision 1
# baseline (speedup 1.0000x reference)
import sys
import numpy as np

for _p in ("/opt/trn_rl_repo",):
    if _p not in sys.path:
        sys.path.insert(0, _p)

PATCH = 7
STRIDE = 3
SAMPLE = 64
H_PARAM = 0.5
ORIENT_W = 0.5
OCC_W = 0.05
EPS_NORM = 1e-05
GRID = 126  # (384 - 7)//3 + 1
S = SAMPLE * SAMPLE  # 4096
NCORES = 8
MSH = S // NCORES  # 512 rows per core
KPAD = 3200  # 3136 padded to 25*128

LAST_EXEC_NS = None
DEVICE_OK = False


def _grid_idx(field):
    gx = field[..., 0].reshape(-1)
    gy = field[..., 1].reshape(-1)
    ix = np.clip(np.round((gx + 1.0) * GRID / 2.0 - 0.5).astype(np.int64), 0, GRID - 1)
    iy = np.clip(np.round((gy + 1.0) * GRID / 2.0 - 0.5).astype(np.int64), 0, GRID - 1)
    return iy, ix


def _gather_patches(feat, iy, ix):
    # feat [C, H, W] -> [C*49, S] with torch-unfold channel ordering (c*49 + ki*7+kj)
    C = feat.shape[0]
    n = iy.shape[0]
    by = iy * STRIDE
    bx = ix * STRIDE
    out = np.empty((C, PATCH * PATCH, n), dtype=np.float32)
    for ki in range(PATCH):
        for kj in range(PATCH):
            out[:, ki * PATCH + kj, :] = feat[:, by + ki, bx + kj]
    return out.reshape(C * PATCH * PATCH, n)


def _device_cos(xn_pad, yn_pad):
    """cos = xn_pad.T @ yn_pad, row-sharded over 8 neuroncores."""
    import concourse.bass as bass
    from concourse import mybir
    from concourse.bass_utils import run_bass_kernel_spmd
    from concourse.tile import TileContext

    K = xn_pad.shape[0]
    assert K == KPAD and K % 128 == 0
    KT = K // 128  # 25
    NCH = S // 512  # 8 column chunks

    nc = bass.Bass()
    x_ext = nc.declare_dram_parameter("x", [K, MSH], mybir.dt.float32, isOutput=False)
    y_ext = nc.declare_dram_parameter("y", [K, S], mybir.dt.float32, isOutput=False)
    out_ext = nc.declare_dram_parameter("cos", [MSH, S], mybir.dt.float32, isOutput=True)

    with TileContext(nc) as tc:
        with tc.tile_pool(name="xp", bufs=1) as xp, \
             tc.tile_pool(name="yp", bufs=2) as yp, \
             tc.tile_pool(name="op", bufs=3) as op, \
             tc.tile_pool(name="pp", bufs=4, space="PSUM") as pp:
            x_sb = xp.tile([128, KT, MSH], mybir.dt.float32)
            nc.sync.dma_start(
                out=x_sb, in_=x_ext.rearrange("(kt p) m -> p kt m", p=128)
            )
            for n in range(NCH):
                y_sb = yp.tile([128, KT, 512], mybir.dt.float32)
                nc.sync.dma_start(
                    out=y_sb,
                    in_=y_ext[:, n * 512:(n + 1) * 512].rearrange(
                        "(kt p) s -> p kt s", p=128
                    ),
                )
                for m in range(MSH // 128):
                    ps = pp.tile([128, 512], mybir.dt.float32)
                    for k in range(KT):
                        nc.tensor.matmul(
                            out=ps,
                            lhsT=x_sb[:, k, m * 128:(m + 1) * 128],
                            rhs=y_sb[:, k, :],
                            start=(k == 0),
                            stop=(k == KT - 1),
                        )
                    ot = op.tile([128, 512], mybir.dt.float32)
                    nc.vector.tensor_copy(out=ot, in_=ps)
                    nc.sync.dma_start(
                        out=out_ext[m * 128:(m + 1) * 128, n * 512:(n + 1) * 512],
                        in_=ot,
                    )

    in_maps = [
        {"x": np.ascontiguousarray(xn_pad[:, c * MSH:(c + 1) * MSH]), "y": yn_pad}
        for c in range(NCORES)
    ]
    res = run_bass_kernel_spmd(nc, in_maps, list(range(NCORES)))
    global LAST_EXEC_NS
    LAST_EXEC_NS = getattr(res, "exec_time_ns", None)
    return np.concatenate([res.results[c]["cos"] for c in range(NCORES)], axis=0)


def kernel(target_features, reference_features, target_orient, refer_orient,
           target_field, refer_field):
    global DEVICE_OK
    iy_t, ix_t = _grid_idx(np.asarray(target_field[0], dtype=np.float32))
    iy_r, ix_r = _grid_idx(np.asarray(refer_field[0], dtype=np.float32))

    tf = _gather_patches(np.asarray(target_features[0], np.float32), iy_t, ix_t)
    rf = _gather_patches(np.asarray(reference_features[0], np.float32), iy_r, ix_r)
    to = _gather_patches(np.asarray(target_orient[0], np.float32), iy_t, ix_t)
    ro = _gather_patches(np.asarray(refer_orient[0], np.float32), iy_r, ix_r)

    # cosine normalization (y-mean centering per reference)
    y_mean = rf.mean(axis=1, keepdims=True)
    xc = tf - y_mean
    yc = rf - y_mean
    xn = xc / (np.linalg.norm(xc, axis=0, keepdims=True) + EPS_NORM)
    yn = yc / (np.linalg.norm(yc, axis=0, keepdims=True) + EPS_NORM)

    xn_pad = np.zeros((KPAD, S), np.float32)
    xn_pad[:3136] = xn
    yn_pad = np.zeros((KPAD, S), np.float32)
    yn_pad[:3136] = yn

    cos = None
    try:
        cos = _device_cos(xn_pad, yn_pad)
        DEVICE_OK = True
    except Exception as e:  # fall back to host if device path unavailable
        sys.stderr.write(f"device path failed ({type(e).__name__}: {e}); numpy fallback\n")
        cos = xn.T @ yn
    d_total = np.maximum((1.0 - cos) / 2.0, 0.0).astype(np.float32)

    # orientation term: d_or = (X2[s] + Y2[t] - 2*sum_i |dot_i|) / (2*49)
    xs = to.reshape(2, 49, S)
    ys = ro.reshape(2, 49, S)
    X2 = (xs * xs).sum(axis=0)  # [49, S]
    Y2 = (ys * ys).sum(axis=0)
    A = np.zeros((S, S), np.float32)
    for i in range(49):
        A += np.abs(xs[:, i, :].T @ ys[:, i, :])
    d_or = (X2.sum(0)[:, None] + Y2.sum(0)[None, :] - 2.0 * A) / (2.0 * 49)
    np.maximum(d_or, 0.0, out=d_or)
    d_total += ORIENT_W * d_or

    # occurrence penalty
    min_idx = np.argmin(d_total, axis=1)
    counts = np.bincount(min_idx, minlength=S).astype(np.float32)
    norm_factor = d_total.shape[0] / d_total.shape[1]
    d_total += OCC_W * (counts / norm_factor)[None, :]

    # loss
    min_d = d_total.min(axis=1, keepdims=True)
    rel = d_total / (min_d + 1e-05)
    w = np.exp((1.0 - rel) / H_PARAM)
    nw = w / w.sum(axis=1, keepdims=True)
    loss = -np.log(nw.max(axis=1)).mean()
    return np.float32(loss)



# revision 2
# speedup vs baseline: 1.0823x; 1.0823x over previous
import os
import sys
import numpy as np

for _p in ("/opt/trn_rl_repo",):
    if _p not in sys.path:
        sys.path.insert(0, _p)

PATCH = 7
STRIDE = 3
SAMPLE = 64
H_PARAM = 0.5
ORIENT_W = 0.5
OCC_W = 0.05
EPS_NORM = 1e-05
GRID = 126  # (384 - 7)//3 + 1
S = SAMPLE * SAMPLE  # 4096
NCORES = 8
MSH = S // NCORES  # 512 rows per core
KPAD16 = 3200  # 3136 padded to 25*128 (bf16)
KPAD8 = 3328   # 3136 padded to 13*256 (fp8 DoubleRow)
FP8_SCALE = 64.0

VARIANT = os.environ.get("KV2_VARIANT", "fp8")

LAST_EXEC_NS = None
LAST_TRACE = None
DEVICE_OK = False


# ---------------------------------------------------------------------------
# sync legalization: this toolchain's walrus codegen rejects any instruction
# carrying >1 sync wait. Tile emits per-proc minimal waits but is not
# transitive across procs, so provably-implied waits remain; drop those, then
# split any genuine multi-wait (kernel-tail drain) into single-wait spacers.
# ---------------------------------------------------------------------------

def _is_dma_completion(update):
    return update.ant_name.startswith("DMAHW") or update.ant_name.startswith("DMASW")


def elide_redundant_waits(m):
    node_count = [0]

    def new_node():
        node_count[0] += 1
        return node_count[0] - 1

    eng_clock = {}
    sem_hist = {}
    sem_val = {}
    removed = 0
    for fn in m.functions:
        for blk in fn.blocks:
            for inst in blk.instructions:
                eng = str(getattr(inst, "engine", "none"))
                clk = eng_clock.setdefault(eng, set())
                si = getattr(inst, "sync_info", None)
                if si is None:
                    continue
                waits = list(si.on_wait)
                new_waits = []
                for w in waits:
                    if w.wait_reg is not None or w.wait_mode != "sem-ge-imm":
                        new_waits.append(w)
                        continue
                    hist = sem_hist.get(w.id, [])
                    acc = []
                    reached = False
                    for cum, evclk in hist:
                        acc.append(evclk)
                        if cum >= w.wait_value:
                            reached = True
                            break
                    if not reached:
                        new_waits.append(w)
                        continue
                    union = set()
                    for evclk in acc:
                        union |= evclk
                    if union <= clk:
                        removed += 1
                    else:
                        new_waits.append(w)
                        clk |= union
                if len(new_waits) != len(waits):
                    si.on_wait = new_waits
                    inst.sync_info = si
                node = new_node()
                clk.add(node)
                for u in si.on_update:
                    if u.update_reg is not None:
                        continue
                    if u.update_mode == "sem-add-imm":
                        delta = u.update_value
                    elif u.update_mode == "sem-inc":
                        delta = 1
                    else:
                        sem_val[u.id] = u.update_value
                        sem_hist[u.id] = []
                        continue
                    cur = sem_val.get(u.id, 0) + delta
                    sem_val[u.id] = cur
                    if _is_dma_completion(u):
                        ev = frozenset(clk | {new_node()})
                    else:
                        ev = frozenset(clk)
                    sem_hist.setdefault(u.id, []).append((cur, ev))
    return removed


def split_excess_waits(m, max_waits=1):
    from concourse import mybir

    n_split = 0
    for fn in m.functions:
        for blk in fn.blocks:
            out = []
            for inst in blk.instructions:
                si = getattr(inst, "sync_info", None)
                waits = list(si.on_wait) if si is not None else []
                if len(waits) > max_waits:
                    head, tail = waits[:-max_waits], waits[-max_waits:]
                    for i, w in enumerate(head):
                        d = mybir.InstDrain(
                            name=f"{inst.name}w{i}",
                            ins=[],
                            outs=[],
                            bass_is_fusable=False,
                        )
                        d.engine = inst.engine
                        d.sync_info = mybir.SyncInfo(on_wait=[w], on_update=[])
                        out.append(d)
                    si.on_wait = tail
                    inst.sync_info = si
                    n_split += 1
                out.append(inst)
            blk.instructions = out
    return n_split


# ---------------------------------------------------------------------------
# device GEMM: cos = x.T @ y, row-sharded over 8 cores
# ---------------------------------------------------------------------------

def build_gemm(dt_in, kt_elems, dr, apply_passes=True):
    import concourse.bass as bass
    from concourse import mybir
    from concourse.tile import TileContext

    NCH = S // 512
    nc = bass.Bass()
    if dr:
        KT = kt_elems // 256
        xs_shape = [128, KT, 2, MSH]
        ys_shape = [128, KT, 2, 512]
    else:
        KT = kt_elems // 128
        xs_shape = [128, KT, MSH]
        ys_shape = [128, KT, 512]
    x_ext = nc.declare_dram_parameter("x", [kt_elems, MSH], dt_in, isOutput=False)
    y_ext = nc.declare_dram_parameter("y", [kt_elems, S], dt_in, isOutput=False)
    out_ext = nc.declare_dram_parameter("cos", [MSH, S], mybir.dt.float32, isOutput=True)

    def mm(ps, lhsT, rhs, start, stop):
        if dr:
            nc.tensor.matmul(out=ps, lhsT=lhsT, rhs=rhs, start=start, stop=stop,
                             perf_mode=mybir.MatmulPerfMode.DoubleRow)
        else:
            nc.tensor.matmul(out=ps, lhsT=lhsT, rhs=rhs, start=start, stop=stop)

    with TileContext(nc) as tc:
        with tc.tile_pool(name="xp", bufs=1) as xp, \
             tc.tile_pool(name="yp", bufs=2) as yp, \
             tc.tile_pool(name="op", bufs=4) as op, \
             tc.tile_pool(name="sp", bufs=1, space="PSUM") as spp, \
             tc.tile_pool(name="pp", bufs=4, space="PSUM") as pp:
            x_sb = xp.tile(xs_shape, dt_in)
            if dr:
                nc.sync.dma_start(
                    out=x_sb,
                    in_=x_ext.rearrange("(kt i p) m -> p kt i m", p=128, i=2))
            else:
                nc.sync.dma_start(
                    out=x_sb, in_=x_ext.rearrange("(kt p) m -> p kt m", p=128))
            for n in range(NCH):
                y_sb = yp.tile(ys_shape, dt_in)
                if dr:
                    nc.sync.dma_start(
                        out=y_sb,
                        in_=y_ext[:, n * 512:(n + 1) * 512].rearrange(
                            "(kt i p) s -> p kt i s", p=128, i=2))
                else:
                    nc.sync.dma_start(
                        out=y_sb,
                        in_=y_ext[:, n * 512:(n + 1) * 512].rearrange(
                            "(kt p) s -> p kt s", p=128))
                # warmup MM reads only y_sb: absorbs the y-DMA wait on PE so
                # every real matmul carries at most one sync wait
                wps = spp.tile([128, 1], mybir.dt.float32)
                if dr:
                    mm(wps, y_sb[:, 0, :, 0:128], y_sb[:, 0, :, 0:1], True, True)
                else:
                    mm(wps, y_sb[:, 0, 0:128], y_sb[:, 0, 0:1], True, True)
                if n % 2 == 0:
                    ot = op.tile([128, MSH // 128, 2, 512], mybir.dt.float32)
                for m in range(MSH // 128):
                    ps = pp.tile([128, 512], mybir.dt.float32)
                    for k in range(KT):
                        if dr:
                            mm(ps, x_sb[:, k, :, m * 128:(m + 1) * 128],
                               y_sb[:, k, :, :], k == 0, k == KT - 1)
                        else:
                            mm(ps, x_sb[:, k, m * 128:(m + 1) * 128],
                               y_sb[:, k, :], k == 0, k == KT - 1)
                    nc.vector.tensor_copy(out=ot[:, m, n % 2, :], in_=ps)
                if n % 2 == 1:
                    # one batched out-DMA per chunk pair keeps the HWDGE DMA
                    # count <= 8 so no sem-lane recycling waits appear
                    nc.sync.dma_start(
                        out=out_ext[:, (n - 1) * 512:(n + 1) * 512].rearrange(
                            "(m p) (c s) -> p m c s", p=128, s=512),
                        in_=ot)
    if apply_passes:
        elide_redundant_waits(nc.m)
        split_excess_waits(nc.m)
    return nc


def _run_spmd(nc, in_maps):
    from concourse.bass_utils import run_bass_kernel_spmd

    kw = {}
    td = os.environ.get("KV2_TRACEDIR")
    if td:
        os.makedirs(td, exist_ok=True)
        kw["tmpdir"] = td
    res = run_bass_kernel_spmd(nc, in_maps, list(range(NCORES)), **kw)
    global LAST_EXEC_NS, LAST_TRACE
    LAST_EXEC_NS = getattr(res, "exec_time_ns", None)
    it = getattr(res, "instructions_and_trace", None)
    LAST_TRACE = it[1] if it else None
    return res


def _device_cos(xn, yn):
    """cos = xn.T @ yn on 8 neuroncores. xn, yn: [3136, S] fp32."""
    import ml_dtypes
    from concourse import mybir

    if VARIANT == "fp8":
        K = KPAD8
        dt_np = ml_dtypes.float8_e4m3
        nc = build_gemm(mybir.dt.float8e4, K, dr=True)
        scale = FP8_SCALE
    else:
        K = KPAD16
        dt_np = ml_dtypes.bfloat16
        nc = build_gemm(mybir.dt.bfloat16, K, dr=False)
        scale = 1.0

    xq = np.zeros((K, S), dtype=dt_np)
    xq[:3136] = (xn * scale).astype(dt_np)
    yq = np.zeros((K, S), dtype=dt_np)
    yq[:3136] = (yn * scale).astype(dt_np)

    in_maps = [
        {"x": np.ascontiguousarray(xq[:, c * MSH:(c + 1) * MSH]), "y": yq}
        for c in range(NCORES)
    ]
    res = _run_spmd(nc, in_maps)
    cos = np.concatenate([res.results[c]["cos"] for c in range(NCORES)], axis=0)
    if scale != 1.0:
        cos /= scale * scale
    return cos


# ---------------------------------------------------------------------------
# host side
# ---------------------------------------------------------------------------

def _grid_idx(field):
    gx = field[..., 0].reshape(-1)
    gy = field[..., 1].reshape(-1)
    ix = np.clip(np.round((gx + 1.0) * GRID / 2.0 - 0.5).astype(np.int64), 0, GRID - 1)
    iy = np.clip(np.round((gy + 1.0) * GRID / 2.0 - 0.5).astype(np.int64), 0, GRID - 1)
    return iy, ix


def _gather_patches(feat, iy, ix):
    # feat [C, H, W] -> [C*49, S] with torch-unfold channel ordering (c*49 + ki*7+kj)
    C = feat.shape[0]
    n = iy.shape[0]
    by = iy * STRIDE
    bx = ix * STRIDE
    out = np.empty((C, PATCH * PATCH, n), dtype=np.float32)
    for ki in range(PATCH):
        for kj in range(PATCH):
            out[:, ki * PATCH + kj, :] = feat[:, by + ki, bx + kj]
    return out.reshape(C * PATCH * PATCH, n)


def kernel(target_features, reference_features, target_orient, refer_orient,
           target_field, refer_field):
    global DEVICE_OK
    iy_t, ix_t = _grid_idx(np.asarray(target_field[0], dtype=np.float32))
    iy_r, ix_r = _grid_idx(np.asarray(refer_field[0], dtype=np.float32))

    tf = _gather_patches(np.asarray(target_features[0], np.float32), iy_t, ix_t)
    rf = _gather_patches(np.asarray(reference_features[0], np.float32), iy_r, ix_r)
    to = _gather_patches(np.asarray(target_orient[0], np.float32), iy_t, ix_t)
    ro = _gather_patches(np.asarray(refer_orient[0], np.float32), iy_r, ix_r)

    # cosine normalization (y-mean centering per reference)
    y_mean = rf.mean(axis=1, keepdims=True)
    xc = tf - y_mean
    yc = rf - y_mean
    xn = xc / (np.linalg.norm(xc, axis=0, keepdims=True) + EPS_NORM)
    yn = yc / (np.linalg.norm(yc, axis=0, keepdims=True) + EPS_NORM)

    cos = None
    try:
        cos = _device_cos(xn, yn)
        DEVICE_OK = True
    except Exception as e:  # fall back to host if device path unavailable
        sys.stderr.write(f"device path failed ({type(e).__name__}: {e}); numpy fallback\n")
        cos = xn.T @ yn
    d_total = np.maximum((1.0 - cos) / 2.0, 0.0).astype(np.float32)

    # orientation term: d_or = (X2[s] + Y2[t] - 2*sum_i |dot_i|) / (2*49)
    xs = to.reshape(2, 49, S)
    ys = ro.reshape(2, 49, S)
    X2 = (xs * xs).sum(axis=0)  # [49, S]
    Y2 = (ys * ys).sum(axis=0)
    A = np.zeros((S, S), np.float32)
    for i in range(49):
        A += np.abs(xs[:, i, :].T @ ys[:, i, :])
    d_or = (X2.sum(0)[:, None] + Y2.sum(0)[None, :] - 2.0 * A) / (2.0 * 49)
    np.maximum(d_or, 0.0, out=d_or)
    d_total += ORIENT_W * d_or

    # occurrence penalty
    min_idx = np.argmin(d_total, axis=1)
    counts = np.bincount(min_idx, minlength=S).astype(np.float32)
    norm_factor = d_total.shape[0] / d_total.shape[1]
    d_total += OCC_W * (counts / norm_factor)[None, :]

    # loss
    min_d = d_total.min(axis=1, keepdims=True)
    rel = d_total / (min_d + 1e-05)
    w = np.exp((1.0 - rel) / H_PARAM)
    nw = w / w.sum(axis=1, keepdims=True)
    loss = -np.log(nw.max(axis=1)).mean()
    return np.float32(loss)
